# revision 18
# baseline (speedup 1.0000x reference)
"""FBPinn (16-window 1D PINN ensemble) forward pass on 8 Trainium2 NeuronCores.

Strategy (MoE-style routing, expert-parallel over windows):
  - Each of the 100k points lies strictly inside only 1-2 of the 16
    overlapping subdomains, so we route: gather the points of each window
    on the host, run each window's small MLP only on its own points, and
    scatter-add the windowed outputs back.
  - Core c owns windows (2c, 2c+1). The two windows are packed into the
    128-partition dimension (64 neurons each) so every matmul runs with
    K=M=128 via block-diagonal weights, and every tanh runs on all 128
    ACT lanes.
  - Layer 1 (per-point affine) is folded into a K=2 outer-product matmul
    with host-folded scales a = W_in/std, c = b_in - W_in*mean/std.
  - Window functions sigmoid((x-l)/s)*sigmoid(-(x-r)/s) are computed on
    device as (1+tanh(x-l))*(1+tanh(r-x))/4 (same table set as the MLP
    tanh), with the 1/4 folded into W_out/b_out on the host.
"""

import numpy as np

# Problem constants (mirrors reference.py static config)
NW = 16
D0, D1 = 0.0, 100.0
OVERLAP = 0.25
SIGMA = 0.5
NEURONS = 64
N = 100_000

NCORES = 8
NPAD = 8192          # per-window padded point count (max real count is 7930)
F = 1024             # chunk width (points) per tanh activation
NCHUNK = NPAD // F
NBLK = NPAD // 512   # 512-wide output blocks per window

_state: dict = {}


def _geometry():
    width = (D1 - D0) / NW
    i = np.arange(NW)
    lo = np.where(i == 0, D0, D0 + (i - OVERLAP / 2) * width)
    hi = np.where(i == NW - 1, D1, D0 + (i + 1 + OVERLAP / 2) * width)
    means = (lo + hi) / 2
    std = (hi - lo) / 2
    ovm = np.empty(NW + 1)
    ovm[0] = lo[0]
    ovm[NW] = hi[-1]
    ovm[1:NW] = (hi[:-1] + lo[1:]) / 2
    f32 = lambda a: np.asarray(a, np.float32)
    return f32(lo), f32(hi), f32(means), f32(std), f32(ovm)


def _build_nc():
    import concourse.bass as bass  # noqa: F401
    import concourse.tile as tile
    from concourse import bacc, mybir

    f32 = mybir.dt.float32
    f32r = mybir.dt.float32r  # 1-pass reduced-precision fp32 matmul
    AF = mybir.ActivationFunctionType
    ALU = mybir.AluOpType

    nc = bacc.Bacc("TRN2", target_bir_lowering=False, debug=False)

    d_xpair = nc.dram_tensor("xpair", [2, NPAD], f32r, kind="ExternalInput")
    d_x32 = nc.dram_tensor("x32", [32, 512], f32, kind="ExternalInput")
    d_lin = nc.dram_tensor("lin", [2, 128], f32r, kind="ExternalInput")
    d_lh = nc.dram_tensor("lh", [128, 256], f32r, kind="ExternalInput")
    d_lout = nc.dram_tensor("lout", [128, 2], f32r, kind="ExternalInput")
    d_bv = nc.dram_tensor("bv", [128, 3], f32, kind="ExternalInput")
    d_wb = nc.dram_tensor("wb", [32, 3], f32, kind="ExternalInput")
    d_out = nc.dram_tensor("out", [32, 512], f32, kind="ExternalOutput")

    with tile.TileContext(nc) as tc:
        with (
            tc.tile_pool(name="consts", bufs=1) as cp,
            tc.tile_pool(name="xcp", bufs=3) as xp,
            tc.tile_pool(name="hp", bufs=4) as hp,
            tc.tile_pool(name="pp", bufs=3, space="PSUM") as pp,
            tc.tile_pool(name="pop", bufs=2, space="PSUM") as pop,
            tc.tile_pool(name="mp", bufs=1) as mp,
        ):
            lin = cp.tile([2, 128], f32r, tag="lin")
            nc.sync.dma_start(lin[:], d_lin[:])
            lh = cp.tile([128, 256], f32r, tag="lh")
            nc.sync.dma_start(lh[:], d_lh[:])
            lout = cp.tile([128, 2], f32r, tag="lout")
            nc.sync.dma_start(lout[:], d_lout[:])
            bv = cp.tile([128, 3], f32, tag="bv")
            nc.sync.dma_start(bv[:], d_bv[:])
            x32 = cp.tile([32, 512], f32, tag="x32")
            nc.sync.dma_start(x32[:], d_x32[:])
            wb = cp.tile([32, 3], f32, tag="wb")
            nc.sync.dma_start(wb[:], d_wb[:])

            oacc2 = mp.tile([2, NPAD], f32, tag="oacc2")
            oacc = mp.tile([32, 512], f32, tag="oacc")

            for j in range(NCHUNK):
                xc = xp.tile([2, F], f32r, tag="xc", name=f"xc{j}")
                nc.sync.dma_start(xc[:], d_xpair[:, j * F : (j + 1) * F])

                p1 = pp.tile([128, F], f32, tag="ps", name=f"p1_{j}")
                for s in range(F // 512):
                    sl = slice(s * 512, (s + 1) * 512)
                    nc.tensor.matmul(p1[:, sl], lin[:], xc[:, sl], start=True, stop=True)
                h1 = hp.tile([128, F], f32r, tag="h", name=f"h1_{j}")
                nc.scalar.activation(h1[:], p1[:], AF.Tanh, bias=bv[:, 0:1])

                p2 = pp.tile([128, F], f32, tag="ps", name=f"p2_{j}")
                for s in range(F // 512):
                    sl = slice(s * 512, (s + 1) * 512)
                    nc.tensor.matmul(p2[:, sl], lh[:, 0:128], h1[:, sl], start=True, stop=True)
                h2 = hp.tile([128, F], f32r, tag="h", name=f"h2_{j}")
                nc.scalar.activation(h2[:], p2[:], AF.Tanh, bias=bv[:, 1:2])

                p3 = pp.tile([128, F], f32, tag="ps", name=f"p3_{j}")
                for s in range(F // 512):
                    sl = slice(s * 512, (s + 1) * 512)
                    nc.tensor.matmul(p3[:, sl], lh[:, 128:256], h2[:, sl], start=True, stop=True)
                h3 = hp.tile([128, F], f32r, tag="h", name=f"h3_{j}")
                nc.scalar.activation(h3[:], p3[:], AF.Tanh, bias=bv[:, 2:3])

                for s in range(F // 512):
                    n = j * (F // 512) + s
                    pout = pop.tile([2, 512], f32, tag="po", name=f"po_{n}")
                    nc.tensor.matmul(
                        pout[:], lout[:], h3[:, s * 512 : (s + 1) * 512],
                        start=True, stop=True,
                    )
                    nc.vector.tensor_copy(oacc2[:, n * 512 : (n + 1) * 512], pout[:])
                    # reshuffle into [32, 512] combine layout as soon as ready
                    nc.sync.dma_start(
                        oacc[2 * n : 2 * n + 2, :], oacc2[:, n * 512 : (n + 1) * 512]
                    )

            # Window weights: 4*win = (1+tanh(x-ovm_w)) * (1+tanh(ovm_{w+1}-x))
            wtL = mp.tile([32, 512], f32, tag="wtL")
            nc.scalar.activation(wtL[:], x32[:], AF.Tanh, bias=wb[:, 0:1], scale=1.0)
            wtR = mp.tile([32, 512], f32, tag="wtR")
            nc.scalar.activation(wtR[:], x32[:], AF.Tanh, bias=wb[:, 1:2], scale=-1.0)
            tp = mp.tile([32, 512], f32, tag="tp")
            nc.vector.tensor_scalar_add(tp[:], wtL[:], 1.0)
            win4 = mp.tile([32, 512], f32, tag="win4")
            nc.vector.scalar_tensor_tensor(
                win4[:], wtR[:], 1.0, tp[:], op0=ALU.add, op1=ALU.mult
            )
            fin = mp.tile([32, 512], f32, tag="fin")
            nc.vector.scalar_tensor_tensor(
                fin[:], oacc[:], wb[:, 2:3], win4[:], op0=ALU.add, op1=ALU.mult
            )
            nc.sync.dma_start(d_out[:], fin[:])

    nc.compile()
    return nc


def _get_nc():
    if "nc" not in _state:
        _state["nc"] = _build_nc()
    return _state["nc"]


def _prepare(x, W_in, b_in, W_h, b_h, W_out, b_out):
    x = np.asarray(x, np.float32)
    W_in = np.asarray(W_in, np.float32)
    b_in = np.asarray(b_in, np.float32)
    W_h = np.asarray(W_h, np.float32)
    b_h = np.asarray(b_h, np.float32)
    W_out = np.asarray(W_out, np.float32)
    b_out = np.asarray(b_out, np.float32)

    lo, hi, means, std, ovm = _geometry()

    # ---- host routing: gather each window's points ----
    idxs, counts = [], []
    for w in range(NW):
        idx = np.nonzero((lo[w] < x) & (x < hi[w]))[0]
        assert len(idx) <= NPAD, f"window {w} has {len(idx)} points > NPAD={NPAD}"
        idxs.append(idx)
        counts.append(len(idx))

    in_maps = []
    for c in range(NCORES):
        A, B = 2 * c, 2 * c + 1
        xA = np.full(NPAD, means[A], np.float32)
        xA[: counts[A]] = x[idxs[A]]
        xB = np.full(NPAD, means[B], np.float32)
        xB[: counts[B]] = x[idxs[B]]
        # normalized per-window inputs (matches reference's xn exactly, and
        # keeps f32r matmul operands in [-1.1, 1.1] for precision)
        xpair = np.stack([(xA - means[A]) / std[A], (xB - means[B]) / std[B]])

        # [32, 512]: row 2n = window-A 512-block n, row 2n+1 = window-B block n
        x32 = np.empty((32, 512), np.float32)
        x32[0::2] = xA.reshape(NBLK, 512)
        x32[1::2] = xB.reshape(NBLK, 512)

        lin = np.zeros((2, 128), np.float32)
        lin[0, :64] = W_in[A]
        lin[1, 64:] = W_in[B]

        bv = np.empty((128, 3), np.float32)
        bv[:64, 0] = b_in[A]
        bv[64:, 0] = b_in[B]
        bv[:64, 1] = b_h[0, A]
        bv[64:, 1] = b_h[0, B]
        bv[:64, 2] = b_h[1, A]
        bv[64:, 2] = b_h[1, B]

        lh = np.zeros((128, 256), np.float32)
        lh[:64, 0:64] = W_h[0, A]
        lh[64:, 64:128] = W_h[0, B]
        lh[:64, 128:192] = W_h[1, A]
        lh[64:, 192:256] = W_h[1, B]

        lout = np.zeros((128, 2), np.float32)
        lout[:64, 0] = W_out[A] * 0.25
        lout[64:, 1] = W_out[B] * 0.25

        # wb: col0 = left-edge tanh bias (-ovm_w), col1 = right-edge tanh
        # bias (+ovm_{w+1}), col2 = b_out/4 (per 32-row out layout)
        wb = np.empty((32, 3), np.float32)
        wb[0::2, 0] = -ovm[A]
        wb[1::2, 0] = -ovm[B]
        wb[0::2, 1] = ovm[A + 1]
        wb[1::2, 1] = ovm[B + 1]
        wb[0::2, 2] = b_out[A] * 0.25
        wb[1::2, 2] = b_out[B] * 0.25

        in_maps.append(
            {
                "xpair": xpair,
                "x32": x32,
                "lin": lin,
                "lh": lh,
                "lout": lout,
                "bv": bv,
                "wb": wb,
            }
        )

    return in_maps, idxs, counts


def _postprocess(results, idxs, counts):
    pred = np.zeros(N, np.float32)
    for w in range(NW):
        c, s = divmod(w, 2)
        vals = results[c]["out"][s::2].reshape(NPAD)[: counts[w]]
        pred[idxs[w]] += vals
    return pred


def kernel(x, W_in, b_in, W_h, b_h, W_out, b_out):
    from concourse.bass_utils import run_bass_kernel_spmd

    in_maps, idxs, counts = _prepare(x, W_in, b_in, W_h, b_h, W_out, b_out)
    nc = _get_nc()
    res = run_bass_kernel_spmd(nc, in_maps, core_ids=list(range(NCORES)))
    return _postprocess(res.results, idxs, counts)


# revision 19
# speedup vs baseline: 1.4416x; 1.4416x over previous
"""FBPinn (16-window 1D PINN ensemble) forward pass on 8 Trainium2 NeuronCores.

Strategy (MoE-style routing, expert-parallel over windows):
  - Each of the 100k points lies strictly inside only 1-2 of the 16
    overlapping subdomains, so we route: gather the points of each window
    on the host, run each window's small MLP only on its own points, and
    scatter-add the windowed outputs back.
  - Core c owns windows (2c, 2c+1). The two windows are packed into the
    128-partition dimension (64 neurons each) so every matmul runs with
    K=M=128 via block-diagonal weights, and every tanh runs on all 128
    ACT lanes.
  - Layer 1 (per-point affine) is folded into a K=2 outer-product matmul
    with host-folded scales a = W_in/std, c = b_in - W_in*mean/std.
  - Window functions sigmoid((x-l)/s)*sigmoid(-(x-r)/s) are computed on
    device as (1+tanh(x-l))*(1+tanh(r-x))/4 (same table set as the MLP
    tanh), with the 1/4 folded into W_out/b_out on the host.
"""

import numpy as np

# Problem constants (mirrors reference.py static config)
NW = 16
D0, D1 = 0.0, 100.0
OVERLAP = 0.25
SIGMA = 0.5
NEURONS = 64
N = 100_000

NCORES = 8
NPAD = 8192          # per-window padded point count (max real count is 7930)
F = 1024             # chunk width (points) per tanh activation
NCHUNK = NPAD // F
NBLK = NPAD // 512   # 512-wide output blocks per window

_state: dict = {}


def _geometry():
    width = (D1 - D0) / NW
    i = np.arange(NW)
    lo = np.where(i == 0, D0, D0 + (i - OVERLAP / 2) * width)
    hi = np.where(i == NW - 1, D1, D0 + (i + 1 + OVERLAP / 2) * width)
    means = (lo + hi) / 2
    std = (hi - lo) / 2
    ovm = np.empty(NW + 1)
    ovm[0] = lo[0]
    ovm[NW] = hi[-1]
    ovm[1:NW] = (hi[:-1] + lo[1:]) / 2
    f32 = lambda a: np.asarray(a, np.float32)
    return f32(lo), f32(hi), f32(means), f32(std), f32(ovm)


def _build_nc():
    import concourse.bass as bass  # noqa: F401
    import concourse.tile as tile
    from concourse import bacc, mybir

    f32 = mybir.dt.float32
    f32r = mybir.dt.float32r  # 1-pass reduced-precision fp32 matmul
    AF = mybir.ActivationFunctionType
    ALU = mybir.AluOpType

    nc = bacc.Bacc("TRN2", target_bir_lowering=False, debug=False)

    d_xpair = nc.dram_tensor("xpair", [2, NPAD], f32r, kind="ExternalInput")
    d_x32 = nc.dram_tensor("x32", [32, 512], f32, kind="ExternalInput")
    d_lin = nc.dram_tensor("lin", [2, 128], f32r, kind="ExternalInput")
    d_lh = nc.dram_tensor("lh", [128, 256], f32r, kind="ExternalInput")
    d_lout = nc.dram_tensor("lout", [128, 2], f32r, kind="ExternalInput")
    d_bv = nc.dram_tensor("bv", [128, 3], f32, kind="ExternalInput")
    d_wb = nc.dram_tensor("wb", [32, 3], f32, kind="ExternalInput")
    d_out = nc.dram_tensor("out", [32, 512], f32, kind="ExternalOutput")

    with tile.TileContext(nc) as tc:
        with (
            tc.tile_pool(name="consts", bufs=1) as cp,
            tc.tile_pool(name="xcp", bufs=NCHUNK) as xp,
            tc.tile_pool(name="hp", bufs=NCHUNK) as hp,
            tc.tile_pool(name="pp", bufs=3, space="PSUM") as pp,
            tc.tile_pool(name="pop", bufs=2, space="PSUM") as pop,
            tc.tile_pool(name="mp", bufs=1) as mp,
        ):
            lin = cp.tile([2, 128], f32r, tag="lin")
            nc.sync.dma_start(lin[:], d_lin[:])
            lh = cp.tile([128, 256], f32r, tag="lh")
            nc.sync.dma_start(lh[:], d_lh[:])
            lout = cp.tile([128, 2], f32r, tag="lout")
            nc.sync.dma_start(lout[:], d_lout[:])
            bv = cp.tile([128, 3], f32, tag="bv")
            nc.sync.dma_start(bv[:], d_bv[:])
            x32 = cp.tile([32, 512], f32, tag="x32")
            nc.sync.dma_start(x32[:], d_x32[:])
            wb = cp.tile([32, 3], f32, tag="wb")
            nc.sync.dma_start(wb[:], d_wb[:])

            oacc2 = mp.tile([2, NPAD], f32, tag="oacc2")
            oacc = mp.tile([32, 512], f32, tag="oacc")
            h3big = mp.tile([128, NPAD], f32r, tag="h3big")

            # Window weights early (ACT fills while PE warms up):
            # 4*win = (1+tanh(x-ovm_w)) * (1+tanh(ovm_{w+1}-x))
            wtL = mp.tile([32, 512], f32, tag="wtL")
            nc.scalar.activation(wtL[:], x32[:], AF.Tanh, bias=wb[:, 0:1], scale=1.0)
            wtR = mp.tile([32, 512], f32, tag="wtR")
            nc.scalar.activation(wtR[:], x32[:], AF.Tanh, bias=wb[:, 1:2], scale=-1.0)

            def emit_out(j):
                for s in range(F // 512):
                    n = j * (F // 512) + s
                    pout = pop.tile([2, 512], f32, tag="po", name=f"po_{n}")
                    nc.tensor.matmul(
                        pout[:], lout[:], h3big[:, n * 512 : (n + 1) * 512],
                        start=True, stop=True,
                    )
                    nc.vector.tensor_copy(oacc2[:, n * 512 : (n + 1) * 512], pout[:])
                    # reshuffle into [32, 512] combine layout as soon as ready
                    nc.sync.dma_start(
                        oacc[2 * n : 2 * n + 2, :], oacc2[:, n * 512 : (n + 1) * 512]
                    )

            # ---- layer-major software pipeline ----
            xcs, h1s, h2s = [], [], []
            for j in range(NCHUNK):
                xc = xp.tile([2, F], f32r, tag="xc", name=f"xc{j}")
                nc.sync.dma_start(xc[:], d_xpair[:, j * F : (j + 1) * F])
                xcs.append(xc)

                p1 = pp.tile([128, F], f32, tag="ps", name=f"p1_{j}")
                for s in range(F // 512):
                    sl = slice(s * 512, (s + 1) * 512)
                    nc.tensor.matmul(p1[:, sl], lin[:], xc[:, sl], start=True, stop=True)
                h1 = hp.tile([128, F], f32r, tag="h1", name=f"h1_{j}")
                nc.scalar.activation(h1[:], p1[:], AF.Tanh, bias=bv[:, 0:1])
                h1s.append(h1)

            for j in range(NCHUNK):
                p2 = pp.tile([128, F], f32, tag="ps", name=f"p2_{j}")
                for s in range(F // 512):
                    sl = slice(s * 512, (s + 1) * 512)
                    nc.tensor.matmul(p2[:, sl], lh[:, 0:128], h1s[j][:, sl], start=True, stop=True)
                h2 = hp.tile([128, F], f32r, tag="h2", name=f"h2_{j}")
                nc.scalar.activation(h2[:], p2[:], AF.Tanh, bias=bv[:, 1:2])
                h2s.append(h2)

            for j in range(NCHUNK):
                p3 = pp.tile([128, F], f32, tag="ps", name=f"p3_{j}")
                for s in range(F // 512):
                    sl = slice(s * 512, (s + 1) * 512)
                    nc.tensor.matmul(p3[:, sl], lh[:, 128:256], h2s[j][:, sl], start=True, stop=True)
                nc.scalar.activation(
                    h3big[:, j * F : (j + 1) * F], p3[:], AF.Tanh, bias=bv[:, 2:3]
                )
                if j >= 1:
                    emit_out(j - 1)
            emit_out(NCHUNK - 1)

            tp = mp.tile([32, 512], f32, tag="tp")
            nc.vector.tensor_scalar_add(tp[:], wtL[:], 1.0)
            win4 = mp.tile([32, 512], f32, tag="win4")
            nc.vector.scalar_tensor_tensor(
                win4[:], wtR[:], 1.0, tp[:], op0=ALU.add, op1=ALU.mult
            )
            fin = mp.tile([32, 512], f32, tag="fin")
            nc.vector.scalar_tensor_tensor(
                fin[:], oacc[:], wb[:, 2:3], win4[:], op0=ALU.add, op1=ALU.mult
            )
            nc.sync.dma_start(d_out[:], fin[:])

    nc.compile()
    return nc


def _get_nc():
    if "nc" not in _state:
        _state["nc"] = _build_nc()
    return _state["nc"]


def _prepare(x, W_in, b_in, W_h, b_h, W_out, b_out):
    x = np.asarray(x, np.float32)
    W_in = np.asarray(W_in, np.float32)
    b_in = np.asarray(b_in, np.float32)
    W_h = np.asarray(W_h, np.float32)
    b_h = np.asarray(b_h, np.float32)
    W_out = np.asarray(W_out, np.float32)
    b_out = np.asarray(b_out, np.float32)

    lo, hi, means, std, ovm = _geometry()

    # ---- host routing: gather each window's points ----
    idxs, counts = [], []
    for w in range(NW):
        idx = np.nonzero((lo[w] < x) & (x < hi[w]))[0]
        assert len(idx) <= NPAD, f"window {w} has {len(idx)} points > NPAD={NPAD}"
        idxs.append(idx)
        counts.append(len(idx))

    in_maps = []
    for c in range(NCORES):
        A, B = 2 * c, 2 * c + 1
        xA = np.full(NPAD, means[A], np.float32)
        xA[: counts[A]] = x[idxs[A]]
        xB = np.full(NPAD, means[B], np.float32)
        xB[: counts[B]] = x[idxs[B]]
        # normalized per-window inputs (matches reference's xn exactly, and
        # keeps f32r matmul operands in [-1.1, 1.1] for precision)
        xpair = np.stack([(xA - means[A]) / std[A], (xB - means[B]) / std[B]])

        # [32, 512]: row 2n = window-A 512-block n, row 2n+1 = window-B block n
        x32 = np.empty((32, 512), np.float32)
        x32[0::2] = xA.reshape(NBLK, 512)
        x32[1::2] = xB.reshape(NBLK, 512)

        lin = np.zeros((2, 128), np.float32)
        lin[0, :64] = W_in[A]
        lin[1, 64:] = W_in[B]

        bv = np.empty((128, 3), np.float32)
        bv[:64, 0] = b_in[A]
        bv[64:, 0] = b_in[B]
        bv[:64, 1] = b_h[0, A]
        bv[64:, 1] = b_h[0, B]
        bv[:64, 2] = b_h[1, A]
        bv[64:, 2] = b_h[1, B]

        lh = np.zeros((128, 256), np.float32)
        lh[:64, 0:64] = W_h[0, A]
        lh[64:, 64:128] = W_h[0, B]
        lh[:64, 128:192] = W_h[1, A]
        lh[64:, 192:256] = W_h[1, B]

        lout = np.zeros((128, 2), np.float32)
        lout[:64, 0] = W_out[A] * 0.25
        lout[64:, 1] = W_out[B] * 0.25

        # wb: col0 = left-edge tanh bias (-ovm_w), col1 = right-edge tanh
        # bias (+ovm_{w+1}), col2 = b_out/4 (per 32-row out layout)
        wb = np.empty((32, 3), np.float32)
        wb[0::2, 0] = -ovm[A]
        wb[1::2, 0] = -ovm[B]
        wb[0::2, 1] = ovm[A + 1]
        wb[1::2, 1] = ovm[B + 1]
        wb[0::2, 2] = b_out[A] * 0.25
        wb[1::2, 2] = b_out[B] * 0.25

        in_maps.append(
            {
                "xpair": xpair,
                "x32": x32,
                "lin": lin,
                "lh": lh,
                "lout": lout,
                "bv": bv,
                "wb": wb,
            }
        )

    return in_maps, idxs, counts


def _postprocess(results, idxs, counts):
    pred = np.zeros(N, np.float32)
    for w in range(NW):
        c, s = divmod(w, 2)
        vals = results[c]["out"][s::2].reshape(NPAD)[: counts[w]]
        pred[idxs[w]] += vals
    return pred


def kernel(x, W_in, b_in, W_h, b_h, W_out, b_out):
    from concourse.bass_utils import run_bass_kernel_spmd

    in_maps, idxs, counts = _prepare(x, W_in, b_in, W_h, b_h, W_out, b_out)
    nc = _get_nc()
    res = run_bass_kernel_spmd(nc, in_maps, core_ids=list(range(NCORES)))
    return _postprocess(res.results, idxs, counts)
